# revision 13
# baseline (speedup 1.0000x reference)
"""MoE feed-forward kernel for 8 Trainium2 NeuronCores.

Strategy:
  - Router (tiny: x @ rW, top-2, softmax) runs on host in numpy.
  - Expert-parallel: core e owns routed expert e. Host gathers the tokens
    routed to expert e (padded to the global max capacity C), ships them
    pre-transposed as (D, C); the device runs gelu(x@W1+b1) @ W2 with the
    per-token gate weight folded in on-chip. Host scatter-adds the result.
  - Shared experts: sharded (expert s = core//4, hidden-quarter q = core%4).
    Each core computes its quarter of one shared expert over all tokens;
    host sums the 8 partials (0.5 mean factor folded into sW2 upload).
  - All matmul operands are bf16 (PSUM accumulates fp32): halves DMA bytes
    and enables Fast Weight Load, so LDWEIGHTS hides under the matmuls.
  - Every input is pre-packed on the host into the exact SBUF tile layout
    (partition-major), so each DMA moves 8-16KB contiguous per partition:
    ~128 descriptors per transfer at line rate instead of ~1024 small
    segments at descriptor-overhead rate. Outputs are stored bf16.
  - Routed phase: GEMM1 materializes gelu(x@W1) for all 32 hidden tiles in
    SBUF (bf16), then GEMM2 accumulates all 32 contraction tiles in PSUM
    per output block and gates straight out of PSUM — output stores spread
    across the whole GEMM2 instead of bunching at the end.
  - Only startup-critical loads go upfront; prefetch is ordered behind
    compute-dependent stores/loads on each queue (queues have no
    priorities — order is the only lever). The scalar queue carries no
    DMAs: it is the GELU engine. A warmup block (dummy matmuls + GELU on
    memset data) ramps the PE clock and preloads the activation table
    while the first loads are in flight.
"""

import sys
import types

import numpy as np
import ml_dtypes

sys.path.insert(0, "/opt/trn_rl_repo")

import concourse.bass as bass  # noqa: E402
import concourse.mybir as mybir  # noqa: E402
import concourse.tile as tile  # noqa: E402
from concourse import bacc  # noqa: E402
from concourse.bass_utils import run_bass_kernel_spmd  # noqa: E402

F32 = mybir.dt.float32
BF16 = mybir.dt.bfloat16
NPBF16 = ml_dtypes.bfloat16
GELU = mybir.ActivationFunctionType.Gelu
ADD = mybir.AluOpType.add

D = 1024      # d_model
H = 4096      # expert hidden
HQ = 1024     # shared-expert hidden slice per core (H / 4)
T = 4096      # tokens (2 * 2048)
E = 8         # routed experts
TOP_K = 2
NCORES = 8
NCB = T // 512  # token blocks in the shared phase


def _install_ntff_hook():
    """Shim for the missing antenv.axon_hooks so trace=True can profile."""
    try:
        import antenv
        if "antenv.axon_hooks" in sys.modules:
            return
        mod = types.ModuleType("antenv.axon_hooks")
        mod._hook = None
        mod.set_axon_ntff_profile_hook = lambda h: setattr(mod, "_hook", h)
        mod.get_axon_ntff_profile_hook = lambda: mod._hook
        sys.modules["antenv.axon_hooks"] = mod
        antenv.axon_hooks = mod
        sys.path.insert(0, "/root/.axon_site/trn_agent_boot")
        import trn_boot
        hook = trn_boot._ntff_profile_via_ctypes("/opt/axon/libaxon_pjrt.so")
        mod.set_axon_ntff_profile_hook(hook)
    except Exception:
        pass


def _split_cblocks(c):
    """Split C into token blocks sized {256,384,512} (multiples of 128);
    a lone 128 only if C == 128."""
    blocks = []
    rem = c
    start = 0
    while rem > 0:
        if rem >= 768:
            sz = 512
        elif rem == 640:
            sz = 384
        else:  # 128..512
            sz = rem
        blocks.append((start, sz))
        start += sz
        rem -= sz
    return blocks


def _pack(mat, inner):
    """(R, cols) -> (128, R//128, cols...) partition-major bf16 host packing:
    out[p, a, ...] = mat[a*128 + p, ...]."""
    r = mat.shape[0]
    arr = np.asarray(mat, dtype=NPBF16).reshape(r // 128, 128, *inner)
    return np.ascontiguousarray(np.moveaxis(arr, 1, 0))


_NC_CACHE = {}


def _build_nc(C):
    if C in _NC_CACHE:
        return _NC_CACHE[C]
    CR = C // 128
    cblocks = _split_cblocks(C)

    nc = bacc.Bacc("TRN2", target_bir_lowering=False, debug=False,
                   enable_asserts=True, num_devices=NCORES)

    # all inputs pre-packed host-side to partition-major SBUF layout
    xeT = nc.dram_tensor("xeT", (128, 8, C), BF16, kind="ExternalInput")
    g_d = nc.dram_tensor("g", (CR, 128), F32, kind="ExternalInput")
    W1e = nc.dram_tensor("W1e", (128, 32, 8, 128), BF16, kind="ExternalInput")
    W2e = nc.dram_tensor("W2e", (128, 32, D), BF16, kind="ExternalInput")
    b1e = nc.dram_tensor("b1e", (128, 32), F32, kind="ExternalInput")
    xT = nc.dram_tensor("xT", (128, NCB, 8, 512), BF16, kind="ExternalInput")
    sW1q = nc.dram_tensor("sW1q", (128, 8, 8, 128), BF16, kind="ExternalInput")
    sW2q = nc.dram_tensor("sW2q", (128, 8, D), BF16, kind="ExternalInput")
    sb1q = nc.dram_tensor("sb1q", (128, 8), F32, kind="ExternalInput")
    yr = nc.dram_tensor("yr", (C, D), BF16, kind="ExternalOutput")
    ys = nc.dram_tensor("ys", (T, D), BF16, kind="ExternalOutput")

    with tile.TileContext(nc) as tc:
        # The outermost pool holds everything preloaded across phases. The
        # routed phase runs FIRST: its critical prefix is only ~1.5MB (first
        # xe block + first w1 tiles) vs ~5MB for the shared phase, so the PE
        # starts ~15us earlier; all shared-phase tensors preload during the
        # ~250us routed phase and phase S then runs with zero DMA waits.
        with tc.tile_pool(name="rpre", bufs=1) as rpre:
          w1p = rpre.tile([128, 2, 8, 128], BF16)   # w1 h-tiles 0-1
          w2a = rpre.tile([128, 32, D], BF16)       # full routed W2, resident
          gt = rpre.tile([128, CR], F32)
          b1t = rpre.tile([128, 32], F32)
          sw1 = rpre.tile([128, 8, 8, 128], BF16)   # shared-phase preloads
          sw2 = rpre.tile([128, 8, D], BF16)
          sb1t = rpre.tile([128, 8], F32)
          xs0 = rpre.tile([128, 8, 512], BF16)

          # Warmup while the first loads are in flight: ~10 dummy matmuls on
          # memset data ramp the PE HAM throttle to full clock (~3.4us of
          # sustained activity) and a dummy GELU preloads the scalar-engine
          # activation table, so the first real tiles run at full speed.
          with tc.tile_pool(name="wup", bufs=1) as wup, \
             tc.tile_pool(name="wps", bufs=1, space="PSUM") as wps:
            wt = wup.tile([128, 640], BF16)
            nc.gpsimd.memset(wt[:], 0.0)
            wp = wps.tile([128, 512], F32)
            wo = wup.tile([128, 512], BF16)
            for i in range(10):
                nc.tensor.matmul(wp[:], wt[:, 0:128], wt[:, 128:640],
                                 start=True, stop=True)
            nc.scalar.activation(wo[:], wp[:], GELU)

          # small loads on the scalar queue (sub-512B segments would clog the
          # bulk queues); GELUs only start once the first GEMM lands
          nc.scalar.dma_start(b1t[:], b1e.ap()[:])
          nc.scalar.dma_start(gt[:], g_d.ap().rearrange("a p -> p a")[:])
          nc.scalar.dma_start(sb1t[:], sb1q.ap()[:])
          nc.sync.dma_start(w1p[:], W1e.ap()[:, 0:2, :, :])

          # ---------------- phase R: routed expert -------------------------
          # GEMM1: all 32 hidden tiles -> ht (bf16, resident); GEMM2: per
          # output block accumulate all 32 contraction tiles in PSUM, gate
          # from PSUM, store. Stores spread across the whole GEMM2.
          with tc.tile_pool(name="rxe", bufs=1) as rxe, \
             tc.tile_pool(name="rw1", bufs=2) as rw1, \
             tc.tile_pool(name="rht", bufs=1) as rht, \
             tc.tile_pool(name="rgp", bufs=3) as rgp, \
             tc.tile_pool(name="rph", bufs=2, space="PSUM") as rph, \
             tc.tile_pool(name="rpy", bufs=4, space="PSUM") as rpy:
            xe = rxe.tile([128, 8, C], BF16)
            for (c0, csz) in cblocks:
                nc.gpsimd.dma_start(xe[:, :, c0:c0 + csz],
                                    xeT.ap()[:, :, c0:c0 + csz])
            ht = rht.tile([128, 32, C], BF16)
            yrr = yr.ap().rearrange("(a p) d -> p a d", p=128)
            for h in range(32):
                if h < 2:
                    w1 = w1p[:, h, :, :]
                else:
                    w1t = rw1.tile([128, 8, 128], BF16, tag="w1")
                    nc.sync.dma_start(w1t[:], W1e.ap()[:, h, :, :])
                    w1 = w1t[:]
                if h % 8 == 4:
                    # resident routed-W2 (needed at GEMM2): 2MB per chunk
                    # behind the xe blocks on the gpsimd queue
                    q8 = h // 8
                    nc.gpsimd.dma_start(w2a[:, q8 * 8:(q8 + 1) * 8, :],
                                        W2e.ap()[:, q8 * 8:(q8 + 1) * 8, :])
                for (c0, csz) in cblocks:
                    ph = rph.tile([128, 512], F32, tag="ph")
                    for d in range(8):
                        nc.tensor.matmul(ph[:, :csz],
                                         w1[:, d, :],
                                         xe[:, d, c0:c0 + csz],
                                         start=(d == 0), stop=(d == 7))
                    nc.scalar.activation(ht[:, h, c0:c0 + csz], ph[:, :csz], GELU,
                                         bias=b1t[:, h:h + 1])
            for crow in range(CR):
                if crow == 0:
                    nc.gpsimd.dma_start(w2a[:, 24:32, :], W2e.ap()[:, 24:32, :])
                elif crow == 1:
                    # shared-phase preloads: land mid-GEMM2, long before use
                    nc.sync.dma_start(sw1[:, 0:4, :, :], sW1q.ap()[:, 0:4, :, :])
                elif crow == 2:
                    nc.sync.dma_start(sw1[:, 4:8, :, :], sW1q.ap()[:, 4:8, :, :])
                elif crow == 3:
                    nc.gpsimd.dma_start(sw2[:], sW2q.ap()[:])
                elif crow == 4:
                    nc.gpsimd.dma_start(xs0[:], xT.ap()[:, 0, :, :])
                for dh in range(2):
                    py = rpy.tile([128, 512], F32, tag="py")
                    for h in range(32):
                        nc.tensor.matmul(py[:],
                                         ht[:, h, crow * 128:(crow + 1) * 128],
                                         w2a[:, h, dh * 512:(dh + 1) * 512],
                                         start=(h == 0), stop=(h == 31))
                    yg = rgp.tile([128, 512], BF16, tag="yg")
                    nc.scalar.mul(yg[:], py[:], gt[:, crow:crow + 1])
                    eng = nc.gpsimd if (crow * 2 + dh) % 2 else nc.sync
                    eng.dma_start(yrr[:, crow, dh * 512:(dh + 1) * 512], yg[:])

          # ---------------- phase S: shared-expert slice over all tokens ----
          with tc.tile_pool(name="sxp", bufs=2) as sxp, \
             tc.tile_pool(name="shp", bufs=10) as shp, \
             tc.tile_pool(name="syp", bufs=3) as syp, \
             tc.tile_pool(name="sph", bufs=2, space="PSUM") as sph, \
             tc.tile_pool(name="spy", bufs=4, space="PSUM") as spy:
            ysr = ys.ap().rearrange("(a p) d -> p a d", p=128)
            for cb in range(NCB):
                if cb == 0:
                    xs = xs0
                else:
                    xs = sxp.tile([128, 8, 512], BF16, tag="xs")
                    nc.gpsimd.dma_start(xs[:], xT.ap()[:, cb, :, :])
                hts = []
                for h in range(8):
                    ph = sph.tile([128, 512], F32, tag="ph")
                    for d in range(8):
                        nc.tensor.matmul(ph[:], sw1[:, h, d, :],
                                         xs[:, d, :], start=(d == 0), stop=(d == 7))
                    ht = shp.tile([128, 512], BF16, tag="ht")
                    nc.scalar.activation(ht[:], ph[:], GELU, bias=sb1t[:, h:h + 1])
                    hts.append(ht)
                for cs in range(4):
                    for dh in range(2):
                        py = spy.tile([128, 512], F32, tag="py")
                        for h in range(8):
                            nc.tensor.matmul(py[:], hts[h][:, cs * 128:(cs + 1) * 128],
                                             sw2[:, h, dh * 512:(dh + 1) * 512],
                                             start=(h == 0), stop=(h == 7))
                        yt = syp.tile([128, 512], BF16, tag="yt")
                        nc.vector.tensor_copy(yt[:], py[:])
                        nc.sync.dma_start(ysr[:, cb * 4 + cs, dh * 512:(dh + 1) * 512], yt[:])

    nc.compile()
    nc.finalize()
    _NC_CACHE[C] = nc
    return nc


def _route(xf, rW, rb):
    """Host router: replicates jax top_k (ties -> lower index) + softmax."""
    gates = xf @ rW + rb
    idx = np.argsort(-gates, axis=1, kind="stable")[:, :TOP_K]
    vals = np.take_along_axis(gates, idx, axis=1)
    ex = np.exp(vals - vals[:, :1])
    probs = (ex / ex.sum(axis=1, keepdims=True)).astype(np.float32)
    return idx, probs


def _run(inputs, trace=False):
    x = np.asarray(inputs["x"], dtype=np.float32)
    rW = np.asarray(inputs["rW"], dtype=np.float32)
    rb = np.asarray(inputs["rb"], dtype=np.float32)
    W1 = np.asarray(inputs["W1"], dtype=np.float32)
    b1 = np.asarray(inputs["b1"], dtype=np.float32)
    W2 = np.asarray(inputs["W2"], dtype=np.float32)
    b2 = np.asarray(inputs["b2"], dtype=np.float32)
    sW1 = np.asarray(inputs["sW1"], dtype=np.float32)
    sb1 = np.asarray(inputs["sb1"], dtype=np.float32)
    sW2 = np.asarray(inputs["sW2"], dtype=np.float32)
    sb2 = np.asarray(inputs["sb2"], dtype=np.float32)

    B, L, _ = x.shape
    xf = np.ascontiguousarray(x.reshape(-1, D))
    idx, probs = _route(xf, rW, rb)

    tok = []
    prb = []
    for e in range(E):
        sel = idx == e  # (T, K)
        rows = np.nonzero(sel.any(axis=1))[0]
        p = np.where(sel[rows, 0], probs[rows, 0], probs[rows, 1])
        tok.append(rows)
        prb.append(p.astype(np.float32))
    C = max(128, max((len(r) + 127) // 128 * 128 for r in tok))
    CR = C // 128

    nc = _build_nc(C)

    xfT16 = np.ascontiguousarray(xf.T.astype(NPBF16))       # (D, T)
    # xT packed: [p, cb, a, c] = xf[cb*512+c, a*128+p]
    xT_host = np.ascontiguousarray(
        xfT16.reshape(8, 128, NCB, 512).transpose(1, 2, 0, 3))
    b1_packed = [np.ascontiguousarray(b1[e].reshape(32, 128).T) for e in range(E)]
    in_maps = []
    for core in range(NCORES):
        s, q = core // 4, core % 4
        n_e = len(tok[core])
        xeF = np.zeros((D, C), dtype=NPBF16)
        xeF[:, :n_e] = xfT16[:, tok[core]]
        g = np.zeros((CR, 128), dtype=np.float32)
        g.reshape(-1)[:n_e] = prb[core]
        in_maps.append({
            "xeT": np.ascontiguousarray(np.moveaxis(xeF.reshape(8, 128, C), 1, 0)),
            "g": g,
            "W1e": np.ascontiguousarray(
                W1[core].astype(NPBF16).reshape(8, 128, 32, 128)
                .transpose(1, 2, 0, 3)),
            "W2e": _pack(W2[core], (D,)),
            "b1e": b1_packed[core],
            "xT": xT_host,
            "sW1q": np.ascontiguousarray(
                sW1[s][:, q * HQ:(q + 1) * HQ].astype(NPBF16)
                .reshape(8, 128, 8, 128).transpose(1, 2, 0, 3)),
            "sW2q": _pack(0.5 * sW2[s][q * HQ:(q + 1) * HQ, :], (D,)),
            "sb1q": np.ascontiguousarray(sb1[s][q * HQ:(q + 1) * HQ].reshape(8, 128).T),
        })

    if trace:
        _install_ntff_hook()
    res = run_bass_kernel_spmd(nc, in_maps, list(range(NCORES)), trace=trace)

    out = np.zeros((T, D), dtype=np.float32)
    for core in range(NCORES):
        out += res.results[core]["ys"].astype(np.float32)
    out += 0.5 * (sb2[0] + sb2[1])[None, :]
    for e in range(E):
        n_e = len(tok[e])
        out[tok[e]] += res.results[e]["yr"][:n_e].astype(np.float32)
        out[tok[e]] += prb[e][:, None] * b2[e][None, :]
    return out.reshape(B, L, D).astype(np.float32), res


def kernel(**inputs):
    out, _ = _run(inputs, trace=False)
    return out
